# revision 1
# baseline (speedup 1.0000x reference)
"""Trainium2 Bass kernel for nn_MoE_32332513804634.

MoE: 16 routed experts (top-6, softmax-then-bias routing) + dense shared
expert, T=4096 tokens, D=2048, H=1408, HS=2816, fp32.

Strategy (8 NeuronCores, SPMD):
  - Host computes the gate (cheap: 0.27 GFLOP) and per-expert token lists.
  - Expert parallelism with load-balanced segmentation: expert token lists
    are carved into pieces and packed into uniform per-core "segments"
    (1 big slot of cap 2048 + k small slots of cap 512 per core), so every
    core executes an identical instruction stream over 3584 token slots.
  - Each segment runs SwiGLU for its expert over its gathered tokens with
    bf16 matmuls accumulating in fp32 PSUM (halves HBM traffic vs f32r;
    the kernel is otherwise DMA-bound), the per-token combine weight
    applied as a per-partition DVE scale on the PSUM->SBUF copy.
  - Shared expert is tensor-parallel over its 2816 hidden dim (352 rows
    per core, padded to 384), same pipeline.
  - Host scatters segment outputs back to token rows, sums partials, and
    adds the second-layer biases (cw*b2 per expert, bs2 once) in fp32 --
    this removes ~240 augmented-row matmuls from the device.
"""

import sys
import numpy as np

sys.path.insert(0, "/opt/trn_rl_repo")

import concourse.bass as bass  # noqa: E402
import concourse.tile as tile  # noqa: E402
from concourse import bacc, mybir  # noqa: E402
from concourse.bass_utils import run_bass_kernel_spmd  # noqa: E402

T = 4096
D = 2048
H = 1408
E = 16
TOP_K = 6
HS = 2816
N_CORES = 8
HM = H // 128          # 11
KO = D // 128          # 16
HS_PAD = 384           # shared hidden shard (352) padded to 3*128
HMS = HS_PAD // 128    # 3
BIG_CAP = 2048
SMALL_CAP = 512
F32 = mybir.dt.float32
F32R = mybir.dt.float32r
BF16 = mybir.dt.bfloat16

# matmul operand dtype: bf16 halves HBM traffic (the kernel is DMA-bound in
# f32r) at ~2.5e-3 relative error; accumulation stays fp32 in PSUM.
MM_DT = BF16

_PROGRAM_CACHE: dict = {}


def _to_mm(a):
    if MM_DT == BF16:
        import ml_dtypes
        return np.ascontiguousarray(a).astype(ml_dtypes.bfloat16)
    return np.ascontiguousarray(a)


def _host_gate(xf, gate_w, gate_b):
    """Numpy replica of the reference gate. Returns cw [T, E] dense combine
    weights and per-expert token lists (ascending)."""
    scores = xf @ gate_w.T
    m = scores.max(axis=-1, keepdims=True)
    p = np.exp(scores - m, dtype=np.float32)
    probs = p / p.sum(axis=-1, keepdims=True)
    biased = probs + gate_b
    idx = np.argpartition(biased, E - TOP_K, axis=1)[:, E - TOP_K:]
    mask = np.zeros((xf.shape[0], E), dtype=bool)
    mask[np.arange(xf.shape[0])[:, None], idx] = True
    cw = np.where(mask, probs, 0.0).astype(np.float32)
    toks = [np.flatnonzero(mask[:, e]).astype(np.int64) for e in range(E)]
    return cw, toks


def _plan_segments(counts):
    """Carve expert token counts into pieces and pack into per-core slots.

    Returns (seg_caps, assignment) where seg_caps is the per-core slot
    capacity tuple and assignment[core][slot] = list of (expert, start, n)
    -- here each slot holds exactly one piece (expert, start offset into
    that expert's token list, piece length) or None for an empty slot.
    """
    order = np.argsort(counts)[::-1]
    bigs = []      # (expert, start, n) with n <= BIG_CAP
    smalls = []    # (expert, start, n) with n <= SMALL_CAP
    rema = []      # remainders to chop into smalls
    for i, e in enumerate(order):
        c = int(counts[e])
        if i < N_CORES:
            n = min(c, BIG_CAP)
            bigs.append((int(e), 0, n))
            if c > n:
                rema.append((int(e), n, c - n))
        else:
            rema.append((int(e), 0, c))
    for e, s0, rem in rema:
        o = 0
        while o < rem:
            n = min(SMALL_CAP, rem - o)
            smalls.append((e, s0 + o, n))
            o += n
    n_small_slots = -(-len(smalls) // N_CORES)  # ceil
    seg_caps = (BIG_CAP,) + (SMALL_CAP,) * n_small_slots
    assignment = []
    for c in range(N_CORES):
        slots = [bigs[c]]
        for s in range(n_small_slots):
            k = s * N_CORES + c
            slots.append(smalls[k] if k < len(smalls) else None)
        assignment.append(slots)
    return seg_caps, assignment


def _build_program(seg_caps):
    """Build the SPMD Bass program for the given per-core slot capacities."""
    nc = bacc.Bacc("TRN2", debug=False, num_devices=N_CORES)

    ins = {}
    outs = {}

    def din(name, shape, dt=MM_DT):
        ins[name] = nc.dram_tensor(name, list(shape), dt, kind="ExternalInput").ap()
        return ins[name]

    def dout(name, shape, dt=F32):
        outs[name] = nc.dram_tensor(name, list(shape), dt, kind="ExternalOutput").ap()
        return outs[name]

    for s, cap in enumerate(seg_caps):
        din(f"xg{s}", (D, cap))
        din(f"w1t{s}", (D, H))
        din(f"w3t{s}", (D, H))
        din(f"w2ta{s}", (H, D))
        din(f"b1_{s}", (128, HM), F32)
        din(f"b3_{s}", (128, HM), F32)
        din(f"scl{s}", (128, cap // 128), F32)
        dout(f"oe{s}", (cap, D))
    din("xt", (D, T))
    din("ws1s", (D, HS_PAD))
    din("ws3s", (D, HS_PAD))
    din("ws2sa", (HS_PAD, D))
    din("bs1", (128, HMS), F32)
    din("bs3", (128, HMS), F32)
    dout("zs", (T, D))

    with tile.TileContext(nc) as tc:
        with (
            tc.tile_pool(name="xpool", bufs=2) as xpool,
            tc.tile_pool(name="hpool", bufs=2) as hpool,
            tc.tile_pool(name="wcol", bufs=2) as wcol,
            tc.tile_pool(name="w2pool", bufs=2) as w2pool,
            tc.tile_pool(name="tmp", bufs=2) as tmp,
            tc.tile_pool(name="opool", bufs=3) as opool,
            tc.tile_pool(name="cpool", bufs=1) as cpool,
            tc.tile_pool(name="pp", bufs=2, space="PSUM") as pp,
        ):
            def mlp_segment(xg_ap, w1_ap, w3_ap, w2_ap, b1_ap, b3_ap,
                            scl_ap, out_ap, cap, n_hm, tag, scale_one):
                """One expert segment: out = scale * (swiglu(x) @ W2^T).
                Biases b2/bs2 are added on the host during the combine."""
                n_k2 = n_hm
                x3 = xg_ap.rearrange("(ko p) t -> p ko t", p=128)
                w1c3 = w1_ap.rearrange("(ko p) h -> p ko h", p=128)
                w3c3 = w3_ap.rearrange("(ko p) h -> p ko h", p=128)
                w23 = w2_ap.rearrange("(k p) d -> p k d", p=128)

                b1sb = cpool.tile([128, n_hm], F32, tag=f"b1{tag}")
                b3sb = cpool.tile([128, n_hm], F32, tag=f"b3{tag}")
                nc.sync.dma_start(b1sb[:], b1_ap)
                nc.sync.dma_start(b3sb[:], b3_ap)
                if not scale_one:
                    sclsb = cpool.tile([128, cap // 128], F32, tag=f"scl{tag}")
                    nc.sync.dma_start(sclsb[:], scl_ap)

                n_tc = cap // 512
                for t in range(n_tc):
                    xsb = xpool.tile([128, KO, 512], MM_DT, tag="xg")
                    nc.sync.dma_start(xsb[:], x3[:, :, t * 512:(t + 1) * 512])
                    hsb = hpool.tile([128, HM, 512], MM_DT, tag="h")
                    for hm in range(n_hm):
                        w1t_ = wcol.tile([128, KO, 128], MM_DT, tag="w1c")
                        nc.sync.dma_start(w1t_[:], w1c3[:, :, hm * 128:(hm + 1) * 128])
                        w3t_ = wcol.tile([128, KO, 128], MM_DT, tag="w3c")
                        nc.sync.dma_start(w3t_[:], w3c3[:, :, hm * 128:(hm + 1) * 128])
                        ps1 = pp.tile([128, 512], F32, tag="ph1")
                        for ko in range(KO):
                            nc.tensor.matmul(ps1[:], w1t_[:, ko, :], xsb[:, ko, :],
                                             start=(ko == 0), stop=(ko == KO - 1))
                        ps3 = pp.tile([128, 512], F32, tag="ph3")
                        for ko in range(KO):
                            nc.tensor.matmul(ps3[:], w3t_[:, ko, :], xsb[:, ko, :],
                                             start=(ko == 0), stop=(ko == KO - 1))
                        h1t = tmp.tile([128, 512], F32, tag="h1t")
                        nc.scalar.activation(h1t[:], ps1[:],
                                             mybir.ActivationFunctionType.Silu,
                                             bias=b1sb[:, hm:hm + 1])
                        h3t = tmp.tile([128, 512], F32, tag="h3t")
                        nc.scalar.activation(h3t[:], ps3[:],
                                             mybir.ActivationFunctionType.Identity,
                                             bias=b3sb[:, hm:hm + 1])
                        nc.vector.tensor_mul(hsb[:, hm, :], h1t[:], h3t[:])
                    # second matmul: out rows = tokens
                    for dm in range(4):
                        w2sb = w2pool.tile([128, n_k2, 512], MM_DT, tag="w2s")
                        nc.sync.dma_start(
                            w2sb[:], w23[:, :, dm * 512:(dm + 1) * 512])
                        for tch in range(4):
                            tok0 = t * 512 + tch * 128
                            ps2 = pp.tile([128, 512], F32, tag="po", bufs=4)
                            for k in range(n_k2):
                                lhsT = hsb[:, k, tch * 128:(tch + 1) * 128]
                                nc.tensor.matmul(ps2[:], lhsT,
                                                 w2sb[:, k, :],
                                                 start=(k == 0), stop=(k == n_k2 - 1))
                            osb = opool.tile([128, 512], F32, tag="osb")
                            if scale_one:
                                nc.vector.tensor_copy(osb[:], ps2[:])
                            else:
                                col = tok0 // 128
                                nc.vector.tensor_scalar_mul(
                                    osb[:], ps2[:], sclsb[:, col:col + 1])
                            nc.sync.dma_start(
                                out_ap[tok0:tok0 + 128, dm * 512:(dm + 1) * 512],
                                osb[:])

            for s, cap in enumerate(seg_caps):
                mlp_segment(ins[f"xg{s}"], ins[f"w1t{s}"], ins[f"w3t{s}"],
                            ins[f"w2ta{s}"], ins[f"b1_{s}"], ins[f"b3_{s}"],
                            ins[f"scl{s}"], outs[f"oe{s}"], cap, HM,
                            f"e{s}", False)
            # shared expert (hidden-sharded, all tokens, no combine scale)
            mlp_segment(ins["xt"], ins["ws1s"], ins["ws3s"], ins["ws2sa"],
                        ins["bs1"], ins["bs3"], None, outs["zs"], T, HMS,
                        "sh", True)

    nc.compile()
    return nc


def kernel(x, gate_w, gate_b, w1, b1, w2, b2, w3, b3,
           ws1, bs1, ws2, bs2, ws3, bs3):
    x = np.asarray(x, np.float32)
    xf = np.ascontiguousarray(x.reshape(-1, D))
    gate_w = np.asarray(gate_w, np.float32)
    gate_b = np.asarray(gate_b, np.float32)
    w1 = np.asarray(w1, np.float32)
    b1 = np.asarray(b1, np.float32)
    w2 = np.asarray(w2, np.float32)
    b2 = np.asarray(b2, np.float32)
    w3 = np.asarray(w3, np.float32)
    b3 = np.asarray(b3, np.float32)
    ws1 = np.asarray(ws1, np.float32)
    bs1 = np.asarray(bs1, np.float32)
    ws2 = np.asarray(ws2, np.float32)
    bs2 = np.asarray(bs2, np.float32)
    ws3 = np.asarray(ws3, np.float32)
    bs3 = np.asarray(bs3, np.float32)

    cw, toks = _host_gate(xf, gate_w, gate_b)
    counts = np.array([len(t) for t in toks])
    seg_caps, assignment = _plan_segments(counts)

    if seg_caps not in _PROGRAM_CACHE:
        _PROGRAM_CACHE[seg_caps] = _build_program(seg_caps)
    nc = _PROGRAM_CACHE[seg_caps]

    xT = np.ascontiguousarray(xf.T)  # [D, T]
    xT_mm = _to_mm(xT)

    # per-expert transposed weights (computed once, shared across pieces)
    w1t = {}
    w3t = {}
    w2ta = {}
    need = sorted({p[0] for slots in assignment for p in slots if p is not None})
    for e in need:
        w1t[e] = _to_mm(w1[e].T)
        w3t[e] = _to_mm(w3[e].T)
        w2ta[e] = _to_mm(w2[e].T)

    # shared expert shards
    hs_per = HS // N_CORES  # 352

    in_maps = []
    for c in range(N_CORES):
        m = {}
        for s, cap in enumerate(seg_caps):
            piece = assignment[c][s]
            xg = np.zeros((D, cap), np.float32)
            scl = np.zeros(cap, np.float32)
            if piece is None:
                e = need[0]
                m[f"w1t{s}"] = w1t[e]
                m[f"w3t{s}"] = w3t[e]
                m[f"w2ta{s}"] = w2ta[e]
                m[f"b1_{s}"] = np.zeros((128, HM), np.float32)
                m[f"b3_{s}"] = np.zeros((128, HM), np.float32)
            else:
                e, s0, n = piece
                tk = toks[e][s0:s0 + n]
                xg[:, :n] = xT[:, tk]
                scl[:n] = cw[tk, e]
                m[f"w1t{s}"] = w1t[e]
                m[f"w3t{s}"] = w3t[e]
                m[f"w2ta{s}"] = w2ta[e]
                m[f"b1_{s}"] = np.ascontiguousarray(
                    b1[e].reshape(HM, 128).T)
                m[f"b3_{s}"] = np.ascontiguousarray(
                    b3[e].reshape(HM, 128).T)
            m[f"xg{s}"] = _to_mm(xg)
            m[f"scl{s}"] = np.ascontiguousarray(
                scl.reshape(cap // 128, 128).T)
        # shared shard
        r0 = c * hs_per
        ws1p = np.zeros((D, HS_PAD), np.float32)
        ws1p[:, :hs_per] = ws1[r0:r0 + hs_per].T
        ws3p = np.zeros((D, HS_PAD), np.float32)
        ws3p[:, :hs_per] = ws3[r0:r0 + hs_per].T
        ws2a = np.zeros((HS_PAD, D), np.float32)
        ws2a[:hs_per] = ws2[:, r0:r0 + hs_per].T
        bs1p = np.zeros(HS_PAD, np.float32)
        bs1p[:hs_per] = bs1[r0:r0 + hs_per]
        bs3p = np.zeros(HS_PAD, np.float32)
        bs3p[:hs_per] = bs3[r0:r0 + hs_per]
        m["xt"] = xT_mm
        m["ws1s"] = _to_mm(ws1p)
        m["ws3s"] = _to_mm(ws3p)
        m["ws2sa"] = _to_mm(ws2a)
        m["bs1"] = np.ascontiguousarray(bs1p.reshape(HMS, 128).T)
        m["bs3"] = np.ascontiguousarray(bs3p.reshape(HMS, 128).T)
        in_maps.append(m)

    res = run_bass_kernel_spmd(nc, in_maps, list(range(N_CORES)))

    # host combine: scatter segment outputs + sum shared partials
    y = np.zeros((T, D), np.float32)
    for c in range(N_CORES):
        for s, cap in enumerate(seg_caps):
            piece = assignment[c][s]
            if piece is None:
                continue
            e, s0, n = piece
            tk = toks[e][s0:s0 + n]
            y[tk] += res.results[c][f"oe{s}"][:n]
            y[tk] += cw[tk, e][:, None] * b2[e][None, :]
        y += res.results[c]["zs"]
    y += bs2[None, :]
    return y.reshape(x.shape).astype(np.float32)



# revision 2
# speedup vs baseline: 1.2396x; 1.2396x over previous
"""Trainium2 Bass kernel for nn_MoE_32332513804634.

MoE: 16 routed experts (top-6, softmax-then-bias routing) + dense shared
expert, T=4096 tokens, D=2048, H=1408, HS=2816, fp32.

Strategy (8 NeuronCores, SPMD):
  - Host computes the gate (cheap: 0.27 GFLOP) and per-expert token lists.
  - Uniform slot profile: a small search finds per-core slot sizes
    (multiples of 128, identical across cores -- SPMD needs one program)
    that cover the 16 expert token counts with minimal padding (~4% vs
    the 17% of fixed 2048/512 caps). Each slot is bound per-core to one
    (expert, offset, len) piece; every core runs exactly the same
    instruction stream over sum(profile) routed tokens.
  - Within a slot: x is resident in SBUF, weights stream hm-outer so each
    weight byte is DMA'd once per slot. Layer-2 output is produced
    transposed [D, tokens] (tokens stay on the moving dim -- cost scales
    exactly with tokens, no 128-alignment of token counts needed).
  - Shared expert is token-parallel: each core runs its 512-token slice
    through the full 2816 hidden dim as one extra slot (no padding).
  - bf16 matmul operands + bf16 outputs; fp32 PSUM accumulation. Combine
    weights, b2/bs2 biases, scatter and the 8-way shared sum happen on
    the host in fp32.
"""

import sys
import numpy as np

sys.path.insert(0, "/opt/trn_rl_repo")

import concourse.bass as bass  # noqa: E402
import concourse.tile as tile  # noqa: E402
from concourse import bacc, mybir  # noqa: E402
from concourse.bass_utils import run_bass_kernel_spmd  # noqa: E402

T = 4096
D = 2048
H = 1408
E = 16
TOP_K = 6
HS = 2816
N_CORES = 8
KO = D // 128           # 16
HM = H // 128           # 11
HMS = HS // 128         # 22
SH_TOK = T // N_CORES   # 512 shared-expert tokens per core
MAX_SLOT = 1536
F32 = mybir.dt.float32
BF16 = mybir.dt.bfloat16
MM_DT = BF16

_PROGRAM_CACHE: dict = {}

import ml_dtypes  # noqa: E402

BF16_NP = ml_dtypes.bfloat16


def _host_gate(xf, gate_w, gate_b):
    """Numpy replica of the reference gate. Returns cw [T, E] dense combine
    weights and per-expert token lists (ascending)."""
    scores = xf @ gate_w.T
    m = scores.max(axis=-1, keepdims=True)
    p = np.exp(scores - m, dtype=np.float32)
    probs = p / p.sum(axis=-1, keepdims=True)
    biased = probs + gate_b
    idx = np.argpartition(biased, E - TOP_K, axis=1)[:, E - TOP_K:]
    mask = np.zeros((xf.shape[0], E), dtype=bool)
    mask[np.arange(xf.shape[0])[:, None], idx] = True
    cw = np.where(mask, probs, 0.0).astype(np.float32)
    toks = [np.flatnonzero(mask[:, e]).astype(np.int64) for e in range(E)]
    return cw, toks


def _cover(cs, slots):
    """Greedy cover: for each count (desc) take largest remaining slots until
    covered, then shrink the last taken to the smallest adequate one."""
    if not cs:
        return []
    c = cs[0][1]
    take = []
    s = 0
    for sz in slots:
        if s >= c:
            break
        take.append(sz)
        s += sz
    if s < c:
        return None
    if take:
        need = c - (s - take[-1])
        rest = list(slots)
        for t in take[:-1]:
            rest.remove(t)
        cands = [sz for sz in set(rest) if sz >= need]
        if cands:
            take = take[:-1] + [min(cands)]
    rem = list(slots)
    for t in take:
        rem.remove(t)
    rem.sort(reverse=True)
    sub = _cover(cs[1:], rem)
    if sub is None:
        return None
    return [take] + sub


def _plan_profile(counts):
    """Find per-core slot sizes (desc, multiples of 128, <= MAX_SLOT) common
    to all cores that cover the expert counts with minimal total padding.

    Returns (profile, assignment) where assignment[core][j] is
    (expert, start, n) or None."""
    order = np.argsort(counts)[::-1]
    cs = [(int(e), int(counts[e])) for e in order if counts[e] > 0]

    def parts(n, maxp, maxsz):
        if n == 0:
            yield ()
            return
        if maxp == 0:
            return
        for sz in range(min(n, maxsz), 0, -1):
            for rest in parts(n - sz, maxp - 1, sz):
                yield (sz,) + rest

    found = None
    for total_u in range(-(-sum(counts) // (128 * N_CORES)), 64):
        for prof_u in parts(total_u, 7, MAX_SLOT // 128):
            prof = tuple(s * 128 for s in prof_u)
            slots = sorted(list(prof) * N_CORES, reverse=True)
            r = _cover(cs, slots)
            if r is not None:
                found = (prof, r)
                break
        if found:
            break
    assert found is not None
    prof, groups = found
    # slot instances: per size, list of (core, j)
    inst = {}
    for j, sz in enumerate(prof):
        for c in range(N_CORES):
            inst.setdefault(sz, []).append((c, j))
    assignment = [[None] * len(prof) for _ in range(N_CORES)]
    for (e, cnt), gslots in zip(cs, groups):
        off = 0
        for sz in sorted(gslots, reverse=True):
            c, j = inst[sz].pop()
            n = min(sz, cnt - off)
            if n > 0:
                assignment[c][j] = (e, off, n)
            off += n
    return prof, assignment


def _build_program(profile):
    """SPMD Bass program: len(profile) routed slots + 1 shared slot."""
    nc = bacc.Bacc("TRN2", debug=False, num_devices=N_CORES)

    n_slots = len(profile)
    C = sum(profile) + SH_TOK  # total token columns per core
    ms = max(max(profile), SH_TOK)

    ins = {}

    def din(name, shape, dt=MM_DT):
        ins[name] = nc.dram_tensor(name, list(shape), dt, kind="ExternalInput").ap()
        return ins[name]

    xg = din("xg", (128, KO, C))
    for j, s in enumerate(profile):
        din(f"w1_{j}", (HM, 128, KO, 128))
        din(f"w3_{j}", (HM, 128, KO, 128))
        din(f"w2_{j}", (KO, 128, HM, 128))
        din(f"b1_{j}", (128, HM), F32)
        din(f"b3_{j}", (128, HM), F32)
    din("ws1", (HMS, 128, KO, 128))
    din("ws3", (HMS, 128, KO, 128))
    din("ws2", (KO, 128, HMS, 128))
    din("bs1", (128, HMS), F32)
    din("bs3", (128, HMS), F32)
    oe = nc.dram_tensor("oe", [KO, 128, C], BF16, kind="ExternalOutput").ap()

    with tile.TileContext(nc) as tc:
        with (
            tc.tile_pool(name="xpool", bufs=4) as xpool,
            tc.tile_pool(name="hpool", bufs=1) as hpool,
            tc.tile_pool(name="wcol", bufs=2) as wcol,
            tc.tile_pool(name="w2pool", bufs=2) as w2pool,
            tc.tile_pool(name="tmp", bufs=2) as tmp,
            tc.tile_pool(name="opool", bufs=3) as opool,
            tc.tile_pool(name="cpool", bufs=1) as cpool,
            tc.tile_pool(name="pp", bufs=2, space="PSUM") as pp,
        ):
            def mlp_slot(c0, s, n_hm, w1_ap, w3_ap, w2_ap, b1_ap, b3_ap, tag):
                """out[:, c0:c0+s] = swiglu_mlp(x[:, c0:c0+s]) (no out bias)."""
                chunks = [(t0, min(512, s - t0)) for t0 in range(0, s, 512)]

                b1sb = cpool.tile([128, n_hm], F32, tag=f"b1{tag}")
                b3sb = cpool.tile([128, n_hm], F32, tag=f"b3{tag}")
                nc.sync.dma_start(b1sb[:], b1_ap)
                nc.sync.dma_start(b3sb[:], b3_ap)

                xts = []
                for (t0, n) in chunks:
                    xt = xpool.tile([128, KO, 512], MM_DT, tag="x")
                    nc.sync.dma_start(xt[:, :, :n],
                                      xg[:, :, c0 + t0:c0 + t0 + n])
                    xts.append(xt)

                hsb = hpool.tile([128, HMS, ms], MM_DT, tag="h")
                # ---- layer 1: h = silu(x@W1+b1) * (x@W3+b3), hm-outer ----
                for hm in range(n_hm):
                    w1t = wcol.tile([128, KO, 128], MM_DT, tag="w1c")
                    nc.sync.dma_start(w1t[:], w1_ap[hm])
                    w3t = wcol.tile([128, KO, 128], MM_DT, tag="w3c")
                    nc.sync.dma_start(w3t[:], w3_ap[hm])
                    for ci, (t0, n) in enumerate(chunks):
                        xt = xts[ci]
                        ps1 = pp.tile([128, 512], F32, tag="ps1")
                        for ko in range(KO):
                            nc.tensor.matmul(ps1[:, :n], w1t[:, ko, :],
                                             xt[:, ko, :n],
                                             start=(ko == 0),
                                             stop=(ko == KO - 1))
                        ps3 = pp.tile([128, 512], F32, tag="ps3")
                        for ko in range(KO):
                            nc.tensor.matmul(ps3[:, :n], w3t[:, ko, :],
                                             xt[:, ko, :n],
                                             start=(ko == 0),
                                             stop=(ko == KO - 1))
                        t1 = tmp.tile([128, 512], BF16, tag="t1")
                        nc.scalar.activation(t1[:, :n], ps1[:, :n],
                                             mybir.ActivationFunctionType.Silu,
                                             bias=b1sb[:, hm:hm + 1])
                        t3 = tmp.tile([128, 512], BF16, tag="t3")
                        nc.vector.tensor_scalar_add(t3[:, :n], ps3[:, :n],
                                                    b3sb[:, hm:hm + 1])
                        nc.vector.tensor_mul(hsb[:, hm, t0:t0 + n],
                                             t1[:, :n], t3[:, :n])
                # ---- layer 2: out[d, t] = sum_h W2[d, h] h[h, t] ----
                for dm in range(KO):
                    w2t = w2pool.tile([128, HMS, 128], MM_DT, tag="w2c")
                    nc.sync.dma_start(w2t[:, :n_hm, :], w2_ap[dm])
                    for (t0, n) in chunks:
                        ps2 = pp.tile([128, 512], F32, tag="ps2")
                        for hk in range(n_hm):
                            nc.tensor.matmul(ps2[:, :n], w2t[:, hk, :],
                                             hsb[:, hk, t0:t0 + n],
                                             start=(hk == 0),
                                             stop=(hk == n_hm - 1))
                        ob = opool.tile([128, 512], BF16, tag="ob")
                        nc.vector.tensor_copy(ob[:, :n], ps2[:, :n])
                        nc.sync.dma_start(
                            oe[dm][:, c0 + t0:c0 + t0 + n], ob[:, :n])

            c0 = 0
            for j, s in enumerate(profile):
                mlp_slot(c0, s, HM, ins[f"w1_{j}"], ins[f"w3_{j}"],
                         ins[f"w2_{j}"], ins[f"b1_{j}"], ins[f"b3_{j}"],
                         f"e{j}")
                c0 += s
            mlp_slot(c0, SH_TOK, HMS, ins["ws1"], ins["ws3"], ins["ws2"],
                     ins["bs1"], ins["bs3"], "sh")

    nc.compile()
    return nc


def _pack_w13(w):
    """[H', D] -> [H'/128, 128(d%128), KO, 128(h%128)] bf16 (lhsT tiles)."""
    hm = w.shape[0] // 128
    return np.ascontiguousarray(
        w.reshape(hm, 128, KO, 128).transpose(0, 3, 2, 1)).astype(BF16_NP)


def _pack_w2(w):
    """[D, H'] -> [KO, 128(h%128), H'/128, 128(d%128)] bf16 (lhsT tiles)."""
    hm = w.shape[1] // 128
    return np.ascontiguousarray(
        w.reshape(KO, 128, hm, 128).transpose(0, 3, 2, 1)).astype(BF16_NP)


def kernel(x, gate_w, gate_b, w1, b1, w2, b2, w3, b3,
           ws1, bs1, ws2, bs2, ws3, bs3):
    x = np.asarray(x, np.float32)
    xf = np.ascontiguousarray(x.reshape(-1, D))
    gate_w = np.asarray(gate_w, np.float32)
    gate_b = np.asarray(gate_b, np.float32)
    w1 = np.asarray(w1, np.float32)
    b1 = np.asarray(b1, np.float32)
    w2 = np.asarray(w2, np.float32)
    b2 = np.asarray(b2, np.float32)
    w3 = np.asarray(w3, np.float32)
    b3 = np.asarray(b3, np.float32)
    ws1 = np.asarray(ws1, np.float32)
    bs1 = np.asarray(bs1, np.float32)
    ws2 = np.asarray(ws2, np.float32)
    bs2 = np.asarray(bs2, np.float32)
    ws3 = np.asarray(ws3, np.float32)
    bs3 = np.asarray(bs3, np.float32)

    cw, toks = _host_gate(xf, gate_w, gate_b)
    counts = np.array([len(t) for t in toks])
    profile, assignment = _plan_profile(counts)

    if profile not in _PROGRAM_CACHE:
        _PROGRAM_CACHE[profile] = _build_program(profile)
    nc = _PROGRAM_CACHE[profile]

    C = sum(profile) + SH_TOK
    xT = xf.T  # [D, T] view

    # per-expert packed weights, shared across cores/slots
    need = sorted({p[0] for slots in assignment for p in slots if p is not None})
    w1p = {e: _pack_w13(w1[e]) for e in need}
    w3p = {e: _pack_w13(w3[e]) for e in need}
    w2p = {e: _pack_w2(w2[e]) for e in need}
    b1p = {e: np.ascontiguousarray(b1[e].reshape(HM, 128).T) for e in need}
    b3p = {e: np.ascontiguousarray(b3[e].reshape(HM, 128).T) for e in need}
    zb = np.zeros((128, HM), np.float32)

    ws1p = _pack_w13(ws1)
    ws3p = _pack_w13(ws3)
    ws2p = _pack_w2(ws2)
    bs1p = np.ascontiguousarray(bs1.reshape(HMS, 128).T)
    bs3p = np.ascontiguousarray(bs3.reshape(HMS, 128).T)

    in_maps = []
    for c in range(N_CORES):
        m = {}
        xcols = np.zeros((D, C), np.float32)
        c0 = 0
        for j, s in enumerate(profile):
            piece = assignment[c][j]
            if piece is None:
                e0 = need[0]
                m[f"w1_{j}"] = w1p[e0]
                m[f"w3_{j}"] = w3p[e0]
                m[f"w2_{j}"] = w2p[e0]
                m[f"b1_{j}"] = zb
                m[f"b3_{j}"] = zb
            else:
                e, s0, n = piece
                tk = toks[e][s0:s0 + n]
                xcols[:, c0:c0 + n] = xT[:, tk]
                m[f"w1_{j}"] = w1p[e]
                m[f"w3_{j}"] = w3p[e]
                m[f"w2_{j}"] = w2p[e]
                m[f"b1_{j}"] = b1p[e]
                m[f"b3_{j}"] = b3p[e]
            c0 += s
        xcols[:, c0:c0 + SH_TOK] = xT[:, c * SH_TOK:(c + 1) * SH_TOK]
        m["xg"] = np.ascontiguousarray(
            xcols.reshape(KO, 128, C).transpose(1, 0, 2)).astype(BF16_NP)
        m["ws1"] = ws1p
        m["ws3"] = ws3p
        m["ws2"] = ws2p
        m["bs1"] = bs1p
        m["bs3"] = bs3p
        in_maps.append(m)

    res = run_bass_kernel_spmd(nc, in_maps, list(range(N_CORES)))

    # host combine: scatter slot outputs, apply combine weights + b2, add
    # shared partials + bs2
    y = np.zeros((T, D), np.float32)
    for c in range(N_CORES):
        out = res.results[c]["oe"].astype(np.float32).reshape(D, C)
        c0 = 0
        for j, s in enumerate(profile):
            piece = assignment[c][j]
            if piece is not None:
                e, s0, n = piece
                tk = toks[e][s0:s0 + n]
                cwe = cw[tk, e][:, None]
                y[tk] += cwe * out[:, c0:c0 + n].T
                y[tk] += cwe * b2[e][None, :]
            c0 += s
        y[c * SH_TOK:(c + 1) * SH_TOK] += out[:, c0:c0 + SH_TOK].T
    y += bs2[None, :]
    return y.reshape(x.shape).astype(np.float32)


# revision 5
# speedup vs baseline: 1.3048x; 1.0526x over previous
"""Trainium2 Bass kernel for nn_MoE_32332513804634.

MoE: 16 routed experts (top-6, softmax-then-bias routing) + dense shared
expert, T=4096 tokens, D=2048, H=1408, HS=2816, fp32.

Strategy (8 NeuronCores, SPMD):
  - Host computes the gate (cheap: 0.27 GFLOP) and per-expert token lists.
  - Uniform slot profile: a small search finds per-core slot sizes
    (multiples of 128, identical across cores -- SPMD needs one program)
    that cover the 16 expert token counts with minimal padding (~4% vs
    the 17% of fixed 2048/512 caps). Each slot is bound per-core to one
    (expert, offset, len) piece; every core runs exactly the same
    instruction stream over sum(profile) routed tokens.
  - Within a slot: x is resident in SBUF, weights stream hm-outer so each
    weight byte is DMA'd once per slot. Layer-2 output is produced
    transposed [D, tokens] (tokens stay on the moving dim -- cost scales
    exactly with tokens, no 128-alignment of token counts needed).
  - Shared expert is token-parallel: each core runs its 512-token slice
    through the full 2816 hidden dim as one extra slot (no padding).
  - bf16 matmul operands + bf16 outputs; fp32 PSUM accumulation. Combine
    weights, b2/bs2 biases, scatter and the 8-way shared sum happen on
    the host in fp32.
"""

import sys
import numpy as np

sys.path.insert(0, "/opt/trn_rl_repo")

import concourse.bass as bass  # noqa: E402
import concourse.tile as tile  # noqa: E402
from concourse import bacc, mybir  # noqa: E402
from concourse.bass_utils import run_bass_kernel_spmd  # noqa: E402

T = 4096
D = 2048
H = 1408
E = 16
TOP_K = 6
HS = 2816
N_CORES = 8
KO = D // 128           # 16
HM = H // 128           # 11
HMS = HS // 128         # 22
SH_TOK = T // N_CORES   # 512 shared-expert tokens per core
MAX_SLOT = 1536
F32 = mybir.dt.float32
BF16 = mybir.dt.bfloat16
MM_DT = BF16

_PROGRAM_CACHE: dict = {}

import ml_dtypes  # noqa: E402

BF16_NP = ml_dtypes.bfloat16


def _host_gate(xf, gate_w, gate_b):
    """Numpy replica of the reference gate. Returns cw [T, E] dense combine
    weights and per-expert token lists (ascending)."""
    scores = xf @ gate_w.T
    m = scores.max(axis=-1, keepdims=True)
    p = np.exp(scores - m, dtype=np.float32)
    probs = p / p.sum(axis=-1, keepdims=True)
    biased = probs + gate_b
    idx = np.argpartition(biased, E - TOP_K, axis=1)[:, E - TOP_K:]
    mask = np.zeros((xf.shape[0], E), dtype=bool)
    mask[np.arange(xf.shape[0])[:, None], idx] = True
    cw = np.where(mask, probs, 0.0).astype(np.float32)
    toks = [np.flatnonzero(mask[:, e]).astype(np.int64) for e in range(E)]
    return cw, toks


def _cover(cs, slots):
    """Greedy cover: for each count (desc) take largest remaining slots until
    covered, then shrink the last taken to the smallest adequate one."""
    if not cs:
        return []
    c = cs[0][1]
    take = []
    s = 0
    for sz in slots:
        if s >= c:
            break
        take.append(sz)
        s += sz
    if s < c:
        return None
    if take:
        need = c - (s - take[-1])
        rest = list(slots)
        for t in take[:-1]:
            rest.remove(t)
        cands = [sz for sz in set(rest) if sz >= need]
        if cands:
            take = take[:-1] + [min(cands)]
    rem = list(slots)
    for t in take:
        rem.remove(t)
    rem.sort(reverse=True)
    sub = _cover(cs[1:], rem)
    if sub is None:
        return None
    return [take] + sub


def _plan_profile(counts):
    """Find per-core slot sizes (desc, multiples of 128, <= MAX_SLOT) common
    to all cores that cover the expert counts with minimal total padding.

    Returns (profile, assignment) where assignment[core][j] is
    (expert, start, n) or None."""
    order = np.argsort(counts)[::-1]
    cs = [(int(e), int(counts[e])) for e in order if counts[e] > 0]

    def parts(n, maxp, maxsz):
        if n == 0:
            yield ()
            return
        if maxp == 0:
            return
        for sz in range(min(n, maxsz), 0, -1):
            for rest in parts(n - sz, maxp - 1, sz):
                yield (sz,) + rest

    found = None
    for total_u in range(-(-sum(counts) // (128 * N_CORES)), 64):
        for prof_u in parts(total_u, 7, MAX_SLOT // 128):
            prof = tuple(s * 128 for s in prof_u)
            slots = sorted(list(prof) * N_CORES, reverse=True)
            r = _cover(cs, slots)
            if r is not None:
                found = (prof, r)
                break
        if found:
            break
    assert found is not None
    prof, groups = found
    # interleave big/small slots so every small (weight-DMA-heavy) slot
    # executes right after a big slot whose compute prefetches its weights
    ps = sorted(prof, reverse=True)
    half = (len(ps) + 1) // 2
    inter = []
    for a, b in zip(ps[:half], ps[half:] + [None]):
        inter.append(a)
        if b is not None:
            inter.append(b)
    prof = tuple(inter)
    # slot instances: per size, list of (core, j)
    inst = {}
    for j, sz in enumerate(prof):
        for c in range(N_CORES):
            inst.setdefault(sz, []).append((c, j))
    assignment = [[None] * len(prof) for _ in range(N_CORES)]
    for (e, cnt), gslots in zip(cs, groups):
        off = 0
        for sz in sorted(gslots, reverse=True):
            c, j = inst[sz].pop()
            n = min(sz, cnt - off)
            if n > 0:
                assignment[c][j] = (e, off, n)
            off += n
    return prof, assignment


def _build_program(profile):
    """SPMD Bass program: len(profile) routed slots + 1 shared slot."""
    nc = bacc.Bacc("TRN2", debug=False, num_devices=N_CORES)

    n_slots = len(profile)
    C = sum(profile) + SH_TOK  # total token columns per core
    ms = max(max(profile), SH_TOK)

    ins = {}

    def din(name, shape, dt=MM_DT):
        ins[name] = nc.dram_tensor(name, list(shape), dt, kind="ExternalInput").ap()
        return ins[name]

    xg = din("xg", (128, KO, C))
    for j, s in enumerate(profile):
        din(f"w1_{j}", (HM, 128, KO, 128))
        din(f"w3_{j}", (HM, 128, KO, 128))
        din(f"w2_{j}", (KO, 128, HM, 128))
        din(f"b1_{j}", (128, HM), F32)
        din(f"b3_{j}", (128, HM), F32)
    din("ws1", (HMS, 128, KO, 128))
    din("ws3", (HMS, 128, KO, 128))
    din("ws2", (KO, 128, HMS, 128))
    din("bs1", (128, HMS), F32)
    din("bs3", (128, HMS), F32)
    oe = nc.dram_tensor("oe", [KO, 128, C], BF16, kind="ExternalOutput").ap()

    h_flat = max(HM * max(profile), HMS * SH_TOK)

    with tile.TileContext(nc) as tc:
        with (
            tc.tile_pool(name="xpool", bufs=3) as xpool,
            tc.tile_pool(name="hpool", bufs=1) as hpool,
            tc.tile_pool(name="wcol", bufs=8) as wcol,
            tc.tile_pool(name="w2pool", bufs=4) as w2pool,
            tc.tile_pool(name="tmp", bufs=2) as tmp,
            tc.tile_pool(name="opool", bufs=3) as opool,
            tc.tile_pool(name="cpool", bufs=1) as cpool,
            tc.tile_pool(name="pp", bufs=2, space="PSUM") as pp,
        ):
            def mlp_slot(c0, s, n_hm, w1_ap, w3_ap, w2_ap, b1_ap, b3_ap, tag):
                """out[:, c0:c0+s] = swiglu_mlp(x[:, c0:c0+s]) (no out bias)."""
                chunks = [(t0, min(512, s - t0)) for t0 in range(0, s, 512)]

                b1sb = cpool.tile([128, n_hm], F32, tag=f"b1{tag}")
                b3sb = cpool.tile([128, n_hm], F32, tag=f"b3{tag}")
                nc.sync.dma_start(b1sb[:], b1_ap)
                nc.sync.dma_start(b3sb[:], b3_ap)

                xts = []
                for (t0, n) in chunks:
                    xt = xpool.tile([128, KO, 512], MM_DT, tag="x")
                    nc.sync.dma_start(xt[:, :, :n],
                                      xg[:, :, c0 + t0:c0 + t0 + n])
                    xts.append(xt)

                hsb = hpool.tile([128, h_flat], MM_DT, tag="h")
                # ---- layer 1: h = silu(x@W1+b1) * (x@W3+b3), hm-outer ----
                for hm in range(n_hm):
                    w1t = wcol.tile([128, KO, 128], MM_DT, tag="w1c")
                    nc.sync.dma_start(w1t[:], w1_ap[hm])
                    w3t = wcol.tile([128, KO, 128], MM_DT, tag="w3c")
                    nc.sync.dma_start(w3t[:], w3_ap[hm])
                    for ci, (t0, n) in enumerate(chunks):
                        xt = xts[ci]
                        ps1 = pp.tile([128, 512], F32, tag="ps1")
                        for ko in range(KO):
                            nc.tensor.matmul(ps1[:, :n], w1t[:, ko, :],
                                             xt[:, ko, :n],
                                             start=(ko == 0),
                                             stop=(ko == KO - 1))
                        ps3 = pp.tile([128, 512], F32, tag="ps3")
                        for ko in range(KO):
                            nc.tensor.matmul(ps3[:, :n], w3t[:, ko, :],
                                             xt[:, ko, :n],
                                             start=(ko == 0),
                                             stop=(ko == KO - 1))
                        t1 = tmp.tile([128, 512], BF16, tag="t1")
                        nc.scalar.activation(t1[:, :n], ps1[:, :n],
                                             mybir.ActivationFunctionType.Silu,
                                             bias=b1sb[:, hm:hm + 1])
                        t3 = tmp.tile([128, 512], BF16, tag="t3")
                        nc.vector.tensor_scalar_add(t3[:, :n], ps3[:, :n],
                                                    b3sb[:, hm:hm + 1])
                        ho = hm * s + t0
                        nc.vector.tensor_mul(hsb[:, ho:ho + n],
                                             t1[:, :n], t3[:, :n])
                # ---- layer 2: out[d, t] = sum_h W2[d, h] h[h, t] ----
                for dm in range(KO):
                    w2t = w2pool.tile([128, HMS, 128], MM_DT, tag="w2c")
                    nc.sync.dma_start(w2t[:, :n_hm, :], w2_ap[dm])
                    for (t0, n) in chunks:
                        ps2 = pp.tile([128, 512], F32, tag="ps2")
                        for hk in range(n_hm):
                            ho = hk * s + t0
                            nc.tensor.matmul(ps2[:, :n], w2t[:, hk, :],
                                             hsb[:, ho:ho + n],
                                             start=(hk == 0),
                                             stop=(hk == n_hm - 1))
                        ob = opool.tile([128, 512], BF16, tag="ob")
                        nc.vector.tensor_copy(ob[:, :n], ps2[:, :n])
                        nc.sync.dma_start(
                            oe[dm][:, c0 + t0:c0 + t0 + n], ob[:, :n])

            c0 = 0
            for j, s in enumerate(profile):
                mlp_slot(c0, s, HM, ins[f"w1_{j}"], ins[f"w3_{j}"],
                         ins[f"w2_{j}"], ins[f"b1_{j}"], ins[f"b3_{j}"],
                         f"e{j}")
                c0 += s
            mlp_slot(c0, SH_TOK, HMS, ins["ws1"], ins["ws3"], ins["ws2"],
                     ins["bs1"], ins["bs3"], "sh")

    nc.compile()
    return nc


def _pack_w13(w):
    """[H', D] -> [H'/128, 128(d%128), KO, 128(h%128)] bf16 (lhsT tiles)."""
    hm = w.shape[0] // 128
    return np.ascontiguousarray(
        w.reshape(hm, 128, KO, 128).transpose(0, 3, 2, 1)).astype(BF16_NP)


def _pack_w2(w):
    """[D, H'] -> [KO, 128(h%128), H'/128, 128(d%128)] bf16 (lhsT tiles)."""
    hm = w.shape[1] // 128
    return np.ascontiguousarray(
        w.reshape(KO, 128, hm, 128).transpose(0, 3, 2, 1)).astype(BF16_NP)


def kernel(x, gate_w, gate_b, w1, b1, w2, b2, w3, b3,
           ws1, bs1, ws2, bs2, ws3, bs3):
    x = np.asarray(x, np.float32)
    xf = np.ascontiguousarray(x.reshape(-1, D))
    gate_w = np.asarray(gate_w, np.float32)
    gate_b = np.asarray(gate_b, np.float32)
    w1 = np.asarray(w1, np.float32)
    b1 = np.asarray(b1, np.float32)
    w2 = np.asarray(w2, np.float32)
    b2 = np.asarray(b2, np.float32)
    w3 = np.asarray(w3, np.float32)
    b3 = np.asarray(b3, np.float32)
    ws1 = np.asarray(ws1, np.float32)
    bs1 = np.asarray(bs1, np.float32)
    ws2 = np.asarray(ws2, np.float32)
    bs2 = np.asarray(bs2, np.float32)
    ws3 = np.asarray(ws3, np.float32)
    bs3 = np.asarray(bs3, np.float32)

    cw, toks = _host_gate(xf, gate_w, gate_b)
    counts = np.array([len(t) for t in toks])
    profile, assignment = _plan_profile(counts)

    if profile not in _PROGRAM_CACHE:
        _PROGRAM_CACHE[profile] = _build_program(profile)
    nc = _PROGRAM_CACHE[profile]

    C = sum(profile) + SH_TOK
    xT = xf.T  # [D, T] view

    # per-expert packed weights, shared across cores/slots
    need = sorted({p[0] for slots in assignment for p in slots if p is not None})
    w1p = {e: _pack_w13(w1[e]) for e in need}
    w3p = {e: _pack_w13(w3[e]) for e in need}
    w2p = {e: _pack_w2(w2[e]) for e in need}
    b1p = {e: np.ascontiguousarray(b1[e].reshape(HM, 128).T) for e in need}
    b3p = {e: np.ascontiguousarray(b3[e].reshape(HM, 128).T) for e in need}
    zb = np.zeros((128, HM), np.float32)

    ws1p = _pack_w13(ws1)
    ws3p = _pack_w13(ws3)
    ws2p = _pack_w2(ws2)
    bs1p = np.ascontiguousarray(bs1.reshape(HMS, 128).T)
    bs3p = np.ascontiguousarray(bs3.reshape(HMS, 128).T)

    in_maps = []
    for c in range(N_CORES):
        m = {}
        xcols = np.zeros((D, C), np.float32)
        c0 = 0
        for j, s in enumerate(profile):
            piece = assignment[c][j]
            if piece is None:
                e0 = need[0]
                m[f"w1_{j}"] = w1p[e0]
                m[f"w3_{j}"] = w3p[e0]
                m[f"w2_{j}"] = w2p[e0]
                m[f"b1_{j}"] = zb
                m[f"b3_{j}"] = zb
            else:
                e, s0, n = piece
                tk = toks[e][s0:s0 + n]
                xcols[:, c0:c0 + n] = xT[:, tk]
                m[f"w1_{j}"] = w1p[e]
                m[f"w3_{j}"] = w3p[e]
                m[f"w2_{j}"] = w2p[e]
                m[f"b1_{j}"] = b1p[e]
                m[f"b3_{j}"] = b3p[e]
            c0 += s
        xcols[:, c0:c0 + SH_TOK] = xT[:, c * SH_TOK:(c + 1) * SH_TOK]
        m["xg"] = np.ascontiguousarray(
            xcols.reshape(KO, 128, C).transpose(1, 0, 2)).astype(BF16_NP)
        m["ws1"] = ws1p
        m["ws3"] = ws3p
        m["ws2"] = ws2p
        m["bs1"] = bs1p
        m["bs3"] = bs3p
        in_maps.append(m)

    res = run_bass_kernel_spmd(nc, in_maps, list(range(N_CORES)))

    # host combine: scatter slot outputs, apply combine weights + b2, add
    # shared partials + bs2
    y = np.zeros((T, D), np.float32)
    for c in range(N_CORES):
        out = res.results[c]["oe"].astype(np.float32).reshape(D, C)
        c0 = 0
        for j, s in enumerate(profile):
            piece = assignment[c][j]
            if piece is not None:
                e, s0, n = piece
                tk = toks[e][s0:s0 + n]
                cwe = cw[tk, e][:, None]
                y[tk] += cwe * out[:, c0:c0 + n].T
                y[tk] += cwe * b2[e][None, :]
            c0 += s
        y[c * SH_TOK:(c + 1) * SH_TOK] += out[:, c0:c0 + SH_TOK].T
    y += bs2[None, :]
    return y.reshape(x.shape).astype(np.float32)


# revision 7
# speedup vs baseline: 1.3339x; 1.0223x over previous
"""Trainium2 Bass kernel for nn_MoE_32332513804634.

MoE: 16 routed experts (top-6, softmax-then-bias routing) + dense shared
expert, T=4096 tokens, D=2048, H=1408, HS=2816, fp32.

Strategy (8 NeuronCores, SPMD):
  - Host computes the gate (cheap: 0.27 GFLOP) and per-expert token lists.
  - Uniform slot profile: a small search finds per-core slot sizes
    (multiples of 128, identical across cores -- SPMD needs one program)
    that cover the 16 expert token counts with minimal padding (~4% vs
    the 17% of fixed 2048/512 caps). Each slot is bound per-core to one
    (expert, offset, len) piece; every core runs exactly the same
    instruction stream over sum(profile) routed tokens.
  - Within a slot: x is resident in SBUF, weights stream hm-outer so each
    weight byte is DMA'd once per slot. Layer-2 output is produced
    transposed [D, tokens] (tokens stay on the moving dim -- cost scales
    exactly with tokens, no 128-alignment of token counts needed).
  - Shared expert is token-parallel: each core runs its 512-token slice
    through the full 2816 hidden dim as one extra slot (no padding).
  - bf16 matmul operands + bf16 outputs; fp32 PSUM accumulation. Combine
    weights, b2/bs2 biases, scatter and the 8-way shared sum happen on
    the host in fp32.
"""

import sys
import numpy as np

sys.path.insert(0, "/opt/trn_rl_repo")

import concourse.bass as bass  # noqa: E402
import concourse.tile as tile  # noqa: E402
from concourse import bacc, mybir  # noqa: E402
from concourse.bass_utils import run_bass_kernel_spmd  # noqa: E402

T = 4096
D = 2048
H = 1408
E = 16
TOP_K = 6
HS = 2816
N_CORES = 8
KO = D // 128           # 16
HM = H // 128           # 11
HMS = HS // 128         # 22
SH_TOK = T // N_CORES   # 512 shared-expert tokens per core
MAX_SLOT = 1024
F32 = mybir.dt.float32
BF16 = mybir.dt.bfloat16
MM_DT = BF16

_PROGRAM_CACHE: dict = {}

import ml_dtypes  # noqa: E402

BF16_NP = ml_dtypes.bfloat16


def _host_gate(xf, gate_w, gate_b):
    """Numpy replica of the reference gate. Returns cw [T, E] dense combine
    weights and per-expert token lists (ascending)."""
    scores = xf @ gate_w.T
    m = scores.max(axis=-1, keepdims=True)
    p = np.exp(scores - m, dtype=np.float32)
    probs = p / p.sum(axis=-1, keepdims=True)
    biased = probs + gate_b
    idx = np.argpartition(biased, E - TOP_K, axis=1)[:, E - TOP_K:]
    mask = np.zeros((xf.shape[0], E), dtype=bool)
    mask[np.arange(xf.shape[0])[:, None], idx] = True
    cw = np.where(mask, probs, 0.0).astype(np.float32)
    toks = [np.flatnonzero(mask[:, e]).astype(np.int64) for e in range(E)]
    return cw, toks


def _cover(cs, slots):
    """Greedy cover: for each count (desc) take largest remaining slots until
    covered, then shrink the last taken to the smallest adequate one."""
    if not cs:
        return []
    c = cs[0][1]
    take = []
    s = 0
    for sz in slots:
        if s >= c:
            break
        take.append(sz)
        s += sz
    if s < c:
        return None
    if take:
        need = c - (s - take[-1])
        rest = list(slots)
        for t in take[:-1]:
            rest.remove(t)
        cands = [sz for sz in set(rest) if sz >= need]
        if cands:
            take = take[:-1] + [min(cands)]
    rem = list(slots)
    for t in take:
        rem.remove(t)
    rem.sort(reverse=True)
    sub = _cover(cs[1:], rem)
    if sub is None:
        return None
    return [take] + sub


def _plan_profile(counts):
    """Find per-core slot sizes (desc, multiples of 128, <= MAX_SLOT) common
    to all cores that cover the expert counts with minimal total padding.

    Returns (profile, assignment) where assignment[core][j] is
    (expert, start, n) or None."""
    order = np.argsort(counts)[::-1]
    cs = [(int(e), int(counts[e])) for e in order if counts[e] > 0]

    def parts(n, maxp, maxsz):
        if n == 0:
            yield ()
            return
        if maxp == 0:
            return
        for sz in range(min(n, maxsz), 0, -1):
            for rest in parts(n - sz, maxp - 1, sz):
                yield (sz,) + rest

    found = None
    for total_u in range(-(-sum(counts) // (128 * N_CORES)), 64):
        for prof_u in parts(total_u, 7, MAX_SLOT // 128):
            prof = tuple(s * 128 for s in prof_u)
            slots = sorted(list(prof) * N_CORES, reverse=True)
            r = _cover(cs, slots)
            if r is not None:
                found = (prof, r)
                break
        if found:
            break
    assert found is not None
    prof, groups = found
    # interleave big/small slots so every small (weight-DMA-heavy) slot
    # executes right after a big slot whose compute prefetches its weights
    ps = sorted(prof, reverse=True)
    half = (len(ps) + 1) // 2
    inter = []
    for a, b in zip(ps[:half], ps[half:] + [None]):
        inter.append(a)
        if b is not None:
            inter.append(b)
    prof = tuple(inter)
    # slot instances: per size, list of (core, j)
    inst = {}
    for j, sz in enumerate(prof):
        for c in range(N_CORES):
            inst.setdefault(sz, []).append((c, j))
    assignment = [[None] * len(prof) for _ in range(N_CORES)]
    for (e, cnt), gslots in zip(cs, groups):
        off = 0
        for sz in sorted(gslots, reverse=True):
            c, j = inst[sz].pop()
            n = min(sz, cnt - off)
            if n > 0:
                assignment[c][j] = (e, off, n)
            off += n
    return prof, assignment


def _build_program(profile):
    """SPMD Bass program: len(profile) routed slots + 1 shared slot."""
    nc = bacc.Bacc("TRN2", debug=False, num_devices=N_CORES)

    n_slots = len(profile)
    C = sum(profile) + SH_TOK  # total token columns per core
    ms = max(max(profile), SH_TOK)

    ins = {}

    def din(name, shape, dt=MM_DT):
        ins[name] = nc.dram_tensor(name, list(shape), dt, kind="ExternalInput").ap()
        return ins[name]

    xg = din("xg", (128, KO, C))
    for j, s in enumerate(profile):
        din(f"w1_{j}", (HM, 128, KO, 128))
        din(f"w3_{j}", (HM, 128, KO, 128))
        din(f"w2_{j}", (KO, 128, HM, 128))
        din(f"b1_{j}", (128, HM), F32)
        din(f"b3_{j}", (128, HM), F32)
    din("ws1", (HMS, 128, KO, 128))
    din("ws3", (HMS, 128, KO, 128))
    din("ws2", (KO, 128, HMS, 128))
    din("bs1", (128, HMS), F32)
    din("bs3", (128, HMS), F32)
    oe = nc.dram_tensor("oe", [KO, 128, C], BF16, kind="ExternalOutput").ap()

    h_flat = max(HM * max(profile), HMS * SH_TOK)

    with tile.TileContext(nc) as tc:
        with (
            tc.tile_pool(name="xpool", bufs=4) as xpool,
            tc.tile_pool(name="hpool", bufs=1) as hpool,
            tc.tile_pool(name="wcol", bufs=11) as wcol,
            tc.tile_pool(name="w2pool", bufs=3) as w2pool,
            tc.tile_pool(name="tmp", bufs=2) as tmp,
            tc.tile_pool(name="opool", bufs=4) as opool,
            tc.tile_pool(name="cpool", bufs=1) as cpool,
            tc.tile_pool(name="pp", bufs=2, space="PSUM") as pp,
        ):
            # warm the PE (and its HAM clock gate) with throwaway matmuls
            # while the first slot's x/weight DMAs are in flight
            wsc1 = cpool.tile([128, 128], MM_DT, tag="wsc1")
            wsc2 = cpool.tile([128, 512], MM_DT, tag="wsc2")
            nc.vector.memset(wsc1[:], 0)
            nc.vector.memset(wsc2[:], 0)
            for _ in range(32):
                pw = pp.tile([128, 512], F32, tag="ps1")
                nc.tensor.matmul(pw[:], wsc1[:], wsc2[:], start=True,
                                 stop=True)
            def mlp_slot(c0, s, n_hm, w1_ap, w3_ap, w2_ap, b1_ap, b3_ap, tag):
                """out[:, c0:c0+s] = swiglu_mlp(x[:, c0:c0+s]) (no out bias)."""
                chunks = [(t0, min(512, s - t0)) for t0 in range(0, s, 512)]

                b1sb = cpool.tile([128, n_hm], F32, tag=f"b1{tag}")
                b3sb = cpool.tile([128, n_hm], F32, tag=f"b3{tag}")
                nc.sync.dma_start(b1sb[:], b1_ap)
                nc.sync.dma_start(b3sb[:], b3_ap)

                xts = []
                for (t0, n) in chunks:
                    xt = xpool.tile([128, KO, 512], MM_DT, tag="x")
                    nc.sync.dma_start(xt[:, :, :n],
                                      xg[:, :, c0 + t0:c0 + t0 + n])
                    xts.append(xt)

                hsb = hpool.tile([128, h_flat], MM_DT, tag="h")
                # ---- layer 1: h = silu(x@W1+b1) * (x@W3+b3), hm-outer ----
                for hm in range(n_hm):
                    w1t = wcol.tile([128, KO, 128], MM_DT, tag="w1c")
                    nc.sync.dma_start(w1t[:], w1_ap[hm])
                    w3t = wcol.tile([128, KO, 128], MM_DT, tag="w3c")
                    nc.sync.dma_start(w3t[:], w3_ap[hm])
                    for ci, (t0, n) in enumerate(chunks):
                        xt = xts[ci]
                        ps1 = pp.tile([128, 512], F32, tag="ps1")
                        for ko in range(KO):
                            nc.tensor.matmul(ps1[:, :n], w1t[:, ko, :],
                                             xt[:, ko, :n],
                                             start=(ko == 0),
                                             stop=(ko == KO - 1))
                        ps3 = pp.tile([128, 512], F32, tag="ps3")
                        for ko in range(KO):
                            nc.tensor.matmul(ps3[:, :n], w3t[:, ko, :],
                                             xt[:, ko, :n],
                                             start=(ko == 0),
                                             stop=(ko == KO - 1))
                        t1 = tmp.tile([128, 512], BF16, tag="t1")
                        nc.scalar.activation(t1[:, :n], ps1[:, :n],
                                             mybir.ActivationFunctionType.Silu,
                                             bias=b1sb[:, hm:hm + 1])
                        t3 = tmp.tile([128, 512], BF16, tag="t3")
                        nc.vector.tensor_scalar_add(t3[:, :n], ps3[:, :n],
                                                    b3sb[:, hm:hm + 1])
                        ho = hm * s + t0
                        nc.vector.tensor_mul(hsb[:, ho:ho + n],
                                             t1[:, :n], t3[:, :n])
                # ---- layer 2: out[d, t] = sum_h W2[d, h] h[h, t] ----
                for dm in range(KO):
                    w2t = w2pool.tile([128, HMS, 128], MM_DT, tag="w2c")
                    nc.sync.dma_start(w2t[:, :n_hm, :], w2_ap[dm])
                    for (t0, n) in chunks:
                        ps2 = pp.tile([128, 512], F32, tag="ps2")
                        for hk in range(n_hm):
                            ho = hk * s + t0
                            nc.tensor.matmul(ps2[:, :n], w2t[:, hk, :],
                                             hsb[:, ho:ho + n],
                                             start=(hk == 0),
                                             stop=(hk == n_hm - 1))
                        ob = opool.tile([128, 512], BF16, tag="ob")
                        nc.vector.tensor_copy(ob[:, :n], ps2[:, :n])
                        nc.sync.dma_start(
                            oe[dm][:, c0 + t0:c0 + t0 + n], ob[:, :n])

            c0 = 0
            for j, s in enumerate(profile):
                mlp_slot(c0, s, HM, ins[f"w1_{j}"], ins[f"w3_{j}"],
                         ins[f"w2_{j}"], ins[f"b1_{j}"], ins[f"b3_{j}"],
                         f"e{j}")
                c0 += s
            mlp_slot(c0, SH_TOK, HMS, ins["ws1"], ins["ws3"], ins["ws2"],
                     ins["bs1"], ins["bs3"], "sh")

    nc.compile()
    return nc


def _pack_w13(w):
    """[H', D] -> [H'/128, 128(d%128), KO, 128(h%128)] bf16 (lhsT tiles)."""
    hm = w.shape[0] // 128
    return np.ascontiguousarray(
        w.reshape(hm, 128, KO, 128).transpose(0, 3, 2, 1)).astype(BF16_NP)


def _pack_w2(w):
    """[D, H'] -> [KO, 128(h%128), H'/128, 128(d%128)] bf16 (lhsT tiles)."""
    hm = w.shape[1] // 128
    return np.ascontiguousarray(
        w.reshape(KO, 128, hm, 128).transpose(0, 3, 2, 1)).astype(BF16_NP)


def kernel(x, gate_w, gate_b, w1, b1, w2, b2, w3, b3,
           ws1, bs1, ws2, bs2, ws3, bs3):
    x = np.asarray(x, np.float32)
    xf = np.ascontiguousarray(x.reshape(-1, D))
    gate_w = np.asarray(gate_w, np.float32)
    gate_b = np.asarray(gate_b, np.float32)
    w1 = np.asarray(w1, np.float32)
    b1 = np.asarray(b1, np.float32)
    w2 = np.asarray(w2, np.float32)
    b2 = np.asarray(b2, np.float32)
    w3 = np.asarray(w3, np.float32)
    b3 = np.asarray(b3, np.float32)
    ws1 = np.asarray(ws1, np.float32)
    bs1 = np.asarray(bs1, np.float32)
    ws2 = np.asarray(ws2, np.float32)
    bs2 = np.asarray(bs2, np.float32)
    ws3 = np.asarray(ws3, np.float32)
    bs3 = np.asarray(bs3, np.float32)

    cw, toks = _host_gate(xf, gate_w, gate_b)
    counts = np.array([len(t) for t in toks])
    profile, assignment = _plan_profile(counts)

    if profile not in _PROGRAM_CACHE:
        _PROGRAM_CACHE[profile] = _build_program(profile)
    nc = _PROGRAM_CACHE[profile]

    C = sum(profile) + SH_TOK
    xT = xf.T  # [D, T] view

    # per-expert packed weights, shared across cores/slots
    need = sorted({p[0] for slots in assignment for p in slots if p is not None})
    w1p = {e: _pack_w13(w1[e]) for e in need}
    w3p = {e: _pack_w13(w3[e]) for e in need}
    w2p = {e: _pack_w2(w2[e]) for e in need}
    b1p = {e: np.ascontiguousarray(b1[e].reshape(HM, 128).T) for e in need}
    b3p = {e: np.ascontiguousarray(b3[e].reshape(HM, 128).T) for e in need}
    zb = np.zeros((128, HM), np.float32)

    ws1p = _pack_w13(ws1)
    ws3p = _pack_w13(ws3)
    ws2p = _pack_w2(ws2)
    bs1p = np.ascontiguousarray(bs1.reshape(HMS, 128).T)
    bs3p = np.ascontiguousarray(bs3.reshape(HMS, 128).T)

    in_maps = []
    for c in range(N_CORES):
        m = {}
        xcols = np.zeros((D, C), np.float32)
        c0 = 0
        for j, s in enumerate(profile):
            piece = assignment[c][j]
            if piece is None:
                e0 = need[0]
                m[f"w1_{j}"] = w1p[e0]
                m[f"w3_{j}"] = w3p[e0]
                m[f"w2_{j}"] = w2p[e0]
                m[f"b1_{j}"] = zb
                m[f"b3_{j}"] = zb
            else:
                e, s0, n = piece
                tk = toks[e][s0:s0 + n]
                xcols[:, c0:c0 + n] = xT[:, tk]
                m[f"w1_{j}"] = w1p[e]
                m[f"w3_{j}"] = w3p[e]
                m[f"w2_{j}"] = w2p[e]
                m[f"b1_{j}"] = b1p[e]
                m[f"b3_{j}"] = b3p[e]
            c0 += s
        xcols[:, c0:c0 + SH_TOK] = xT[:, c * SH_TOK:(c + 1) * SH_TOK]
        m["xg"] = np.ascontiguousarray(
            xcols.reshape(KO, 128, C).transpose(1, 0, 2)).astype(BF16_NP)
        m["ws1"] = ws1p
        m["ws3"] = ws3p
        m["ws2"] = ws2p
        m["bs1"] = bs1p
        m["bs3"] = bs3p
        in_maps.append(m)

    res = run_bass_kernel_spmd(nc, in_maps, list(range(N_CORES)))

    # host combine: scatter slot outputs, apply combine weights + b2, add
    # shared partials + bs2
    y = np.zeros((T, D), np.float32)
    for c in range(N_CORES):
        out = res.results[c]["oe"].astype(np.float32).reshape(D, C)
        c0 = 0
        for j, s in enumerate(profile):
            piece = assignment[c][j]
            if piece is not None:
                e, s0, n = piece
                tk = toks[e][s0:s0 + n]
                cwe = cw[tk, e][:, None]
                y[tk] += cwe * out[:, c0:c0 + n].T
                y[tk] += cwe * b2[e][None, :]
            c0 += s
        y[c * SH_TOK:(c + 1) * SH_TOK] += out[:, c0:c0 + SH_TOK].T
    y += bs2[None, :]
    return y.reshape(x.shape).astype(np.float32)
